# revision 2
# baseline (speedup 1.0000x reference)
"""NestedMLP MoE-routed kernel for 8 TRN2 NeuronCores.

Strategy:
  - Host routes tokens by expert (argsort of expert_mask), splits each
    expert's tokens across the 8 cores (data-parallel), pads each
    per-core expert group to a common capacity so all cores run one SPMD
    program.
  - Activations are kept feature-major ("transposed", [feature, token])
    so both matmuls are natural lhsT.T @ rhs with the contraction dim on
    partitions, and the per-feature biases are per-partition (fusable
    into the ACT/DVE PSUM eviction).
  - Weights/activations are bf16 (f32 PSUM accumulation); biases are f32;
    the output is staged/stored bf16 and upcast to f32 on the host.
  - Per expert e (shift = 3-e): d_in = 1024>>shift, d_hid = 4*d_in,
    d_out = 1024>>shift, using the nested weight slices
    w1[:d_hid,:d_in], w2[:d_out,:d_hid].
  - Weight/x DMAs are emitted in first-use order at <=1MB col-aligned
    granularity, so every tile lands just-in-time with ~3x bandwidth
    margin (the problem is a compute/DMA "ridge": compute-per-byte is
    the same for every expert, so coarse groups arrive exactly at the
    deadline and any hiccup stalls the PE).
  - Compute issue order is software-pipelined one chunk deep
    (mm1[c+1] issues before mm2[c]) so the PE never waits on the
    ACT-gelu latency at chunk/expert boundaries; ht uses two ping-pong
    buffers sized for the worst adjacent pair (e2+e3).
"""

import math
import sys
import types

sys.path.insert(0, "/opt/trn_rl_repo")

import ml_dtypes
import numpy as np

P = 128
E = 4
D = 1024
H = 4096
OUT = 1024
NCORES = 8
MLP_RATIO = 4

BF16 = ml_dtypes.bfloat16

# (d_in, d_hid, d_out) per expert
DIMS = [((D >> (E - 1 - e)), (D >> (E - 1 - e)) * MLP_RATIO, (OUT >> (E - 1 - e))) for e in range(E)]
CHUNK_W = 512  # token columns per matmul pass; 512 = one PSUM bank of f32
FIRST_CHUNK = 64  # tiny bootstrap chunk so the first matmul needs minimal DMA


def _round_up(v, m):
    return ((v + m - 1) // m) * m


def _tile_fmajor(a2d):
    """[F, C] -> [128, F//128, C] with row f = po*128 + pi."""
    f, c = a2d.shape
    return np.ascontiguousarray(a2d.reshape(f // P, P, c).transpose(1, 0, 2))


def _sp(cn):
    """warm per-MM issue spacing (ns) for an N=cn matmul."""
    return cn / 2.4 + 2.5


def _build_graph(caps):
    """Build the SPMD Bass graph for per-core per-expert capacities `caps`."""
    import concourse.mybir as mybir
    import concourse.tile as tile
    from concourse import bacc

    f32 = mybir.dt.float32
    bf16 = mybir.dt.bfloat16
    Gelu = mybir.ActivationFunctionType.Gelu

    ctot = sum(caps)
    offs = np.concatenate([[0], np.cumsum(caps)]).astype(int)

    nc = bacc.Bacc(None, target_bir_lowering=False, debug=False)
    xt_d = nc.declare_dram_parameter("xt", [P, D // P, ctot], bf16, isOutput=False)
    w1_d = nc.declare_dram_parameter("w1t", [P, D // P, H], bf16, isOutput=False)
    w2_d = nc.declare_dram_parameter("w2t", [P, H // P, OUT], bf16, isOutput=False)
    b1_d = nc.declare_dram_parameter("b1t", [P, H // P], f32, isOutput=False)
    b2_d = nc.declare_dram_parameter("b2t", [P, OUT // P], f32, isOutput=False)
    y_d = nc.declare_dram_parameter("yt", [P, OUT // P, ctot], bf16, isOutput=True)

    def chunk_plan(e):
        plan, c0 = [], 0
        if e == 0:
            plan.append((0, min(FIRST_CHUNK, caps[0])))
            c0 = plan[-1][1]
        while c0 < caps[e]:
            cn = min(CHUNK_W, caps[e] - c0)
            plan.append((c0, cn))
            c0 += cn
        return plan

    # flat chunk list + per-chunk mm1/mm2 issue costs, then an approximate
    # PE-issue timeline under the 1-deep software pipeline
    # (mm1_0, mm1_1, mm2_0, mm1_2, mm2_1, ..., mm2_last)
    chunks = []  # (e, c0, cn)
    for e in range(E):
        for c0, cn in chunk_plan(e):
            chunks.append((e, c0, cn))
    nch = len(chunks)

    def costs(i):
        e, c0, cn = chunks[i]
        d_in, d_hid, d_out = DIMS[e]
        c1 = (d_hid // P) * (d_in // P) * _sp(cn)
        c2 = (d_out // P) * (d_hid // P) * _sp(cn)
        return c1, c2

    T1 = [0.0] * nch  # issue time of mm1 block of chunk i
    T2 = [0.0] * nch  # issue time of mm2 block of chunk i
    t = costs(0)[0]
    T1[0] = 0.0
    for i in range(1, nch):
        T1[i] = t
        t += costs(i)[0]
        T2[i - 1] = t
        t += costs(i - 1)[1]
    T2[nch - 1] = t

    first_chunk_of = {}
    for i, (e, c0, cn) in enumerate(chunks):
        first_chunk_of.setdefault(e, i)

    with tile.TileContext(nc) as tc:
        with (
            tc.tile_pool(name="wpool", bufs=1) as wpool,
            tc.tile_pool(name="xpool", bufs=1) as xpool,
            tc.tile_pool(name="hpool", bufs=1) as hpool,
            tc.tile_pool(name="ypool", bufs=2) as ypool,
            tc.tile_pool(name="pspool", bufs=8, space="PSUM") as pspool,
        ):
            # PE warm-up: dependency-free dummy matmuls keep the PE busy from
            # ~t=6.4us until the first real matmul's inputs land, so the HAM
            # clock gate reaches K=8/8 (2.4 GHz) without a cold restart.
            wu = wpool.tile([P, P], bf16, tag="warmup")
            nc.vector.memset(wu[:], 0.0)
            wact = wpool.tile([P, P], bf16, tag="warmact")
            # dummy activation: loads the ACT Gelu table before the first
            # real gelu needs it (table load is ~1.3us)
            nc.scalar.activation(wact[:], wu[:], Gelu, bias=0.0)

            def warm_mms(n):
                for _ in range(n):
                    wps = pspool.tile([P, P], f32, tag="ps")
                    nc.tensor.matmul(wps[:], wu[:], wu[:], start=True, stop=True)

            b1sb = wpool.tile([P, H // P], f32, tag="b1")
            b2sb = wpool.tile([P, OUT // P], f32, tag="b2")

            # ---- DMA plan: emit every input transfer in first-use order ----
            # Weight pieces are col-aligned blocks of <=~1MB so a piece's
            # completion unblocks exactly the m-tiles that read it.
            w1x = {}  # k -> list of (lo, hi, k0, tile)
            w2x = {}
            xts = {}  # (e, c0) -> tile

            items = []  # (need_ns, seq, emit_fn)
            seq = [0]

            def add(need, fn):
                items.append((need, seq[0], fn))
                seq[0] += 1

            def emit_w_piece(xdict, dram, k0, k1, lo, hi, tagp):
                t_ = wpool.tile([P, k1 - k0, hi - lo], bf16, tag=f"{tagp}_{k0}_{lo}")
                nc.sync.dma_start(t_[:], dram[:, k0:k1, lo:hi])
                for k in range(k0, k1):
                    xdict.setdefault(k, []).append((lo, hi, k0, t_))

            def plan_w(e, which):
                """pieces (k0,k1,lo,hi,first_use_mtile) newly needed by expert e."""
                nk_of = (lambda i: DIMS[i][0] // P) if which == 1 else (lambda i: DIMS[i][1] // P)
                ncols_of = (lambda i: DIMS[i][1]) if which == 1 else (lambda i: DIMS[i][2])
                nk_prev = nk_of(e - 1) if e > 0 else 0
                cols_prev = ncols_of(e - 1) if e > 0 else 0
                nk, cols = nk_of(e), ncols_of(e)
                out = []
                # ~1MB blocks, 128-aligned
                def blocks(k0, k1, lo0, hi0):
                    bytes_per_col = (k1 - k0) * P * 2
                    bc = max(P, min(1024, (1 << 20) // bytes_per_col // P * P))
                    b = lo0
                    while b < hi0:
                        hi = min(b + bc, hi0)
                        out.append((k0, k1, b, hi, b // P))
                        b = hi
                if nk > nk_prev:
                    blocks(nk_prev, nk, 0, cols)
                if nk_prev and cols > cols_prev:
                    blocks(0, nk_prev, cols_prev, cols)
                return out

            # bootstrap: first matmul needs only w1[k0,m0] + xt(e0,c0); order
            # these (plus the cheap biases) ahead of everything else.
            add(-4.0, lambda: emit_w_piece(w1x, w1_d, 0, 1, 0, P, "w1"))
            e0, c00, cn0 = chunks[0]
            def emit_xt(i):
                e, c0, cn = chunks[i]
                nk1 = DIMS[e][0] // P
                xt = xpool.tile([P, nk1, cn], bf16, tag=f"xt_{e}_{c0}")
                nc.sync.dma_start(xt[:], xt_d[:, :nk1, offs[e] + c0 : offs[e] + c0 + cn])
                xts[(e, c0)] = xt
            add(-3.0, lambda: emit_xt(0))
            add(-2.0, lambda: nc.sync.dma_start(b1sb[:], b1_d[:]))
            add(-1.0, lambda: nc.sync.dma_start(b2sb[:], b2_d[:]))

            for i in range(1, nch):
                add(T1[i], (lambda j: (lambda: emit_xt(j)))(i))

            for e in range(E):
                d_in, d_hid, d_out = DIMS[e]
                nk1 = d_in // P
                nk2 = d_hid // P
                i0 = first_chunk_of[e]
                cn_first = chunks[i0][2]
                for (k0, k1, lo, hi, mfirst) in plan_w(e, 1):
                    if e == 0 and k0 == 0 and lo == 0:
                        # skip the bootstrap piece already emitted
                        lo = P
                        if lo >= hi:
                            continue
                        mfirst = 1
                    need = T1[i0] + mfirst * nk1 * _sp(cn_first)
                    add(need, (lambda a, b_, c, d_: (lambda: emit_w_piece(w1x, w1_d, a, b_, c, d_, "w1")))(k0, k1, lo, hi))
                for (k0, k1, lo, hi, mfirst) in plan_w(e, 2):
                    need = T2[i0] + mfirst * nk2 * _sp(cn_first)
                    add(need, (lambda a, b_, c, d_: (lambda: emit_w_piece(w2x, w2_d, a, b_, c, d_, "w2")))(k0, k1, lo, hi))

            for _, _, fn in sorted(items, key=lambda it: (it[0], it[1])):
                fn()

            def wslice(xdict, k, m):
                """[128, 128] lhsT slice for feature cols [m*128,(m+1)*128)."""
                lo_c, hi_c = m * P, (m + 1) * P
                for lo, hi, k0, t_ in xdict[k]:
                    if lo <= lo_c and hi_c <= hi:
                        return t_[:, k - k0, lo_c - lo : hi_c - lo]
                raise AssertionError("weight slice not found")

            # ---- compute: 1-deep software pipeline over chunks ----
            # ht ping-pong buffers sized for the worst adjacent chunk pair
            htA = hpool.tile([P, H // P, CHUNK_W], bf16, tag="htA")  # holds e3 (32 slabs)
            htB = hpool.tile([P, H // P // 2, CHUNK_W], bf16, tag="htB")  # holds e2 (16)

            def mm1_block(i):
                e, c0, cn = chunks[i]
                d_in, d_hid, _ = DIMS[e]
                nk1, nm1 = d_in // P, d_hid // P
                ht = htA if i % 2 == 0 else htB
                xt = xts[(e, c0)]
                for m in range(nm1):
                    ps = pspool.tile([P, cn], f32, tag="ps")
                    for k in range(nk1):
                        nc.tensor.matmul(
                            ps[:],
                            wslice(w1x, k, m),
                            xt[:, k, :],
                            start=(k == 0),
                            stop=(k == nk1 - 1),
                        )
                    nc.scalar.activation(ht[:, m, :cn], ps[:], Gelu, bias=b1sb[:, m : m + 1])

            def mm2_block(i):
                e, c0, cn = chunks[i]
                _, d_hid, d_out = DIMS[e]
                nk2, nm2 = d_hid // P, d_out // P
                ht = htA if i % 2 == 0 else htB
                col = offs[e] + c0
                for m2 in range(nm2):
                    ps = pspool.tile([P, cn], f32, tag="ps")
                    for k2 in range(nk2):
                        nc.tensor.matmul(
                            ps[:],
                            wslice(w2x, k2, m2),
                            ht[:, k2, :cn],
                            start=(k2 == 0),
                            stop=(k2 == nk2 - 1),
                        )
                    # bias-add evicts PSUM to a bf16 SBUF slab; each slab
                    # streams out as soon as it's ready
                    yt = ypool.tile([P, cn], bf16, tag="yt")
                    nc.vector.tensor_scalar_add(yt[:], ps[:], b2sb[:, m2 : m2 + 1])
                    nc.sync.dma_start(y_d[:, m2, col : col + cn], yt[:])

            warm_mms(18)
            mm1_block(0)
            warm_mms(10)
            for i in range(1, nch):
                mm1_block(i)
                mm2_block(i - 1)
            mm2_block(nch - 1)

    nc.compile()
    return nc, ctot, offs


def _ensure_ntff_hook_importable():
    """bass_utils' trace path imports antenv.axon_hooks, which some images
    lack; install a working shim so tracing (e.g. BASS_TRACE=1 in the
    environment) degrades gracefully instead of crashing. No-op when the
    real module exists."""
    try:
        import antenv.axon_hooks  # noqa: F401
        return
    except ImportError:
        pass
    holder = {"hook": None}
    m = types.ModuleType("antenv.axon_hooks")
    m.set_axon_ntff_profile_hook = lambda h: holder.__setitem__("hook", h)
    m.get_axon_ntff_profile_hook = lambda: holder["hook"]
    sys.modules["antenv.axon_hooks"] = m
    try:
        from trn_agent_boot.trn_boot import _ntff_profile_via_ctypes

        m.set_axon_ntff_profile_hook(_ntff_profile_via_ctypes("/opt/axon/libaxon_pjrt.so"))
    except Exception:
        pass  # hook stays None; bass_utils logs and skips tracing


def kernel(x, expert_mask, w1, b1, w2, b2):
    _ensure_ntff_hook_importable()
    from concourse.bass_utils import run_bass_kernel_spmd

    B, N, _ = x.shape
    T = B * N
    xf = np.asarray(x, dtype=np.float32).reshape(T, D)
    mask = np.asarray(expert_mask).reshape(T).astype(np.int64)

    # --- host routing ---
    ids_by_e = [np.nonzero(mask == e)[0] for e in range(E)]
    counts = [len(i) for i in ids_by_e]
    caps = [max(8, _round_up(math.ceil(c / NCORES), 8)) for c in counts]
    # per (core, expert) token id arrays
    core_ids = [[None] * E for _ in range(NCORES)]
    for e in range(E):
        parts = np.array_split(ids_by_e[e], NCORES)
        for c in range(NCORES):
            assert len(parts[c]) <= caps[e]
            core_ids[c][e] = parts[c]

    nc, ctot, offs = _build_graph(caps)

    # --- host input prep ---
    w1t = _tile_fmajor(np.asarray(w1, np.float32).T).astype(BF16)  # [128, 8, H]
    w2t = _tile_fmajor(np.asarray(w2, np.float32).T).astype(BF16)  # [128, 32, OUT]
    b1t = np.ascontiguousarray(np.asarray(b1, np.float32).reshape(H // P, P).T)
    b2t = np.ascontiguousarray(np.asarray(b2, np.float32).reshape(OUT // P, P).T)

    in_maps = []
    for c in range(NCORES):
        xg = np.zeros((ctot, D), np.float32)
        for e in range(E):
            ids = core_ids[c][e]
            xg[offs[e] : offs[e] + len(ids)] = xf[ids]
        xt = _tile_fmajor(xg.T).astype(BF16)  # [128, 8, ctot]
        in_maps.append({"xt": xt, "w1t": w1t, "w2t": w2t, "b1t": b1t, "b2t": b2t})

    res = run_bass_kernel_spmd(nc, in_maps, list(range(NCORES)))

    # --- host output assembly ---
    y = np.zeros((T, OUT), np.float32)
    for c in range(NCORES):
        yr = np.asarray(res.results[c]["yt"]).astype(np.float32)  # [128, 8, ctot]
        yfull = yr.transpose(1, 0, 2).reshape(OUT, ctot)
        for e in range(E):
            d_out = DIMS[e][2]
            ids = core_ids[c][e]
            if len(ids):
                y[ids, :d_out] = yfull[:d_out, offs[e] : offs[e] + len(ids)].T
    return y.reshape(B, N, OUT)


# revision 8
# speedup vs baseline: 1.0141x; 1.0141x over previous
"""NestedMLP MoE-routed kernel for 8 TRN2 NeuronCores.

Strategy:
  - Host routes tokens by expert (argsort of expert_mask), splits each
    expert's tokens across the 8 cores (data-parallel), pads each
    per-core expert group to a common capacity so all cores run one SPMD
    program.
  - Activations are kept feature-major ("transposed", [feature, token])
    so both matmuls are natural lhsT.T @ rhs with the contraction dim on
    partitions, and the per-feature biases are per-partition (fusable
    into the ACT/DVE PSUM eviction).
  - Weights/activations are bf16 (f32 PSUM accumulation); biases are f32;
    the output is staged/stored bf16 and upcast to f32 on the host.
  - Per expert e (shift = 3-e): d_in = 1024>>shift, d_hid = 4*d_in,
    d_out = 1024>>shift, using the nested weight slices
    w1[:d_hid,:d_in], w2[:d_out,:d_hid].
  - Weight/x DMAs are emitted in first-use order at <=1MB col-aligned
    granularity, so every tile lands just-in-time with ~3x bandwidth
    margin (the problem is a compute/DMA "ridge": compute-per-byte is
    the same for every expert, so coarse groups arrive exactly at the
    deadline and any hiccup stalls the PE).
  - Compute issue order is software-pipelined one chunk deep
    (mm1[c+1] issues before mm2[c]) so the PE never waits on the
    ACT-gelu latency at chunk/expert boundaries; ht uses two ping-pong
    buffers sized for the worst adjacent pair (e2+e3).
"""

import math
import sys
import types

sys.path.insert(0, "/opt/trn_rl_repo")

import ml_dtypes
import numpy as np

P = 128
E = 4
D = 1024
H = 4096
OUT = 1024
NCORES = 8
MLP_RATIO = 4

BF16 = ml_dtypes.bfloat16

# (d_in, d_hid, d_out) per expert
DIMS = [((D >> (E - 1 - e)), (D >> (E - 1 - e)) * MLP_RATIO, (OUT >> (E - 1 - e))) for e in range(E)]
CHUNK_W = 512  # token columns per matmul pass; 512 = one PSUM bank of f32
FIRST_CHUNK = 128  # small bootstrap chunk so the first matmul needs minimal DMA


def _round_up(v, m):
    return ((v + m - 1) // m) * m


def _tile_fmajor(a2d):
    """[F, C] -> [128, F//128, C] with row f = po*128 + pi."""
    f, c = a2d.shape
    return np.ascontiguousarray(a2d.reshape(f // P, P, c).transpose(1, 0, 2))


def _sp(cn):
    """warm per-MM issue spacing (ns) for an N=cn matmul."""
    return cn / 2.4 + 2.5


def _build_graph(caps):
    """Build the SPMD Bass graph for per-core per-expert capacities `caps`."""
    import concourse.mybir as mybir
    import concourse.tile as tile
    from concourse import bacc

    f32 = mybir.dt.float32
    bf16 = mybir.dt.bfloat16
    Gelu = mybir.ActivationFunctionType.Gelu

    ctot = sum(caps)
    offs = np.concatenate([[0], np.cumsum(caps)]).astype(int)

    nc = bacc.Bacc(None, target_bir_lowering=False, debug=False)
    xt_d = nc.declare_dram_parameter("xt", [P, D // P, ctot], bf16, isOutput=False)
    w1_d = nc.declare_dram_parameter("w1t", [P, D // P, H], bf16, isOutput=False)
    w2_d = nc.declare_dram_parameter("w2t", [P, H // P, OUT], bf16, isOutput=False)
    b1_d = nc.declare_dram_parameter("b1t", [P, H // P], f32, isOutput=False)
    b2_d = nc.declare_dram_parameter("b2t", [P, OUT // P], f32, isOutput=False)
    y_d = nc.declare_dram_parameter("yt", [P, OUT // P, ctot], bf16, isOutput=True)

    def chunk_plan(e):
        plan, c0 = [], 0
        if e == 0:
            plan.append((0, min(FIRST_CHUNK, caps[0])))
            c0 = plan[-1][1]
        while c0 < caps[e]:
            cn = min(CHUNK_W, caps[e] - c0)
            plan.append((c0, cn))
            c0 += cn
        return plan

    # flat chunk list + per-chunk mm1/mm2 issue costs, then an approximate
    # PE-issue timeline under the 1-deep software pipeline
    # (mm1_0, mm1_1, mm2_0, mm1_2, mm2_1, ..., mm2_last)
    chunks = []  # (e, c0, cn)
    for e in range(E):
        for c0, cn in chunk_plan(e):
            chunks.append((e, c0, cn))
    nch = len(chunks)

    def costs(i):
        e, c0, cn = chunks[i]
        d_in, d_hid, d_out = DIMS[e]
        c1 = (d_hid // P) * (d_in // P) * _sp(cn)
        c2 = (d_out // P) * (d_hid // P) * _sp(cn)
        return c1, c2

    T1 = [0.0] * nch  # issue time of mm1 block of chunk i
    T2 = [0.0] * nch  # issue time of mm2 block of chunk i
    t = costs(0)[0]
    T1[0] = 0.0
    for i in range(1, nch):
        T1[i] = t
        t += costs(i)[0]
        T2[i - 1] = t
        t += costs(i - 1)[1]
    T2[nch - 1] = t

    first_chunk_of = {}
    for i, (e, c0, cn) in enumerate(chunks):
        first_chunk_of.setdefault(e, i)

    with tile.TileContext(nc) as tc:
        with (
            tc.tile_pool(name="wpool", bufs=1) as wpool,
            tc.tile_pool(name="xpool", bufs=1) as xpool,
            tc.tile_pool(name="hpool", bufs=1) as hpool,
            tc.tile_pool(name="ypool", bufs=2) as ypool,
            tc.tile_pool(name="pspool", bufs=8, space="PSUM") as pspool,
        ):
            # PE warm-up: dependency-free dummy matmuls keep the PE busy from
            # ~t=6.4us until the first real matmul's inputs land, so the HAM
            # clock gate reaches K=8/8 (2.4 GHz) without a cold restart.
            wu = wpool.tile([P, P], bf16, tag="warmup")
            nc.vector.memset(wu[:], 0.0)
            wact = wpool.tile([P, P], bf16, tag="warmact")

            def warm_mms(n):
                for _ in range(n):
                    wps = pspool.tile([P, P], f32, tag="ps")
                    nc.tensor.matmul(wps[:], wu[:], wu[:], start=True, stop=True)

            b1sb = wpool.tile([P, H // P], f32, tag="b1")
            b2sb = wpool.tile([P, OUT // P], f32, tag="b2")

            # ---- DMA plan: emit every input transfer in first-use order ----
            # Weight pieces are col-aligned blocks of <=~1MB so a piece's
            # completion unblocks exactly the m-tiles that read it.
            w1x = {}  # k -> list of (lo, hi, k0, tile)
            w2x = {}
            xts = {}  # (e, c0) -> tile

            items = []  # (need_ns, seq, emit_fn)
            seq = [0]

            def add(need, fn):
                items.append((need, seq[0], fn))
                seq[0] += 1

            def emit_w_piece(xdict, dram, k0, k1, lo, hi, tagp, eng=None):
                t_ = wpool.tile([P, k1 - k0, hi - lo], bf16, tag=f"{tagp}_{k0}_{lo}")
                (eng or nc.sync).dma_start(t_[:], dram[:, k0:k1, lo:hi])
                for k in range(k0, k1):
                    xdict.setdefault(k, []).append((lo, hi, k0, t_))

            def plan_w(e, which):
                """pieces (k0,k1,lo,hi,first_use_mtile) newly needed by expert e."""
                nk_of = (lambda i: DIMS[i][0] // P) if which == 1 else (lambda i: DIMS[i][1] // P)
                ncols_of = (lambda i: DIMS[i][1]) if which == 1 else (lambda i: DIMS[i][2])
                nk_prev = nk_of(e - 1) if e > 0 else 0
                cols_prev = ncols_of(e - 1) if e > 0 else 0
                nk, cols = nk_of(e), ncols_of(e)
                out = []
                # ~1MB blocks, 128-aligned
                def blocks(k0, k1, lo0, hi0):
                    bytes_per_col = (k1 - k0) * P * 2
                    bc = max(P, min(1024, (1 << 20) // bytes_per_col // P * P))
                    b = lo0
                    while b < hi0:
                        hi = min(b + bc, hi0)
                        out.append((k0, k1, b, hi, b // P))
                        b = hi
                if nk > nk_prev:
                    blocks(nk_prev, nk, 0, cols)
                if nk_prev and cols > cols_prev:
                    blocks(0, nk_prev, cols_prev, cols)
                return out

            # bootstrap: parallelize the ring-up across the two HW-DGE
            # queues (Sync + Scalar). DMA issue instructions cost ~0.62us
            # each on the issuing engine, so keep e0's pieces coarse and
            # split them across both queues. Sync gets the weights, Scalar
            # gets x/biases (its queue is free until the first gelu).
            def emit_xt(i, eng=None):
                e, c0, cn = chunks[i]
                nk1 = DIMS[e][0] // P
                xt = xpool.tile([P, nk1, cn], bf16, tag=f"xt_{e}_{c0}")
                (eng or nc.sync).dma_start(xt[:], xt_d[:, :nk1, offs[e] + c0 : offs[e] + c0 + cn])
                xts[(e, c0)] = xt
            add(-5.0, lambda: emit_w_piece(w1x, w1_d, 0, 1, 0, DIMS[0][1], "w1"))
            add(-4.9, lambda: emit_xt(0, nc.scalar))
            add(-4.8, lambda: nc.scalar.dma_start(b1sb[:], b1_d[:]))
            add(-4.7, lambda: nc.scalar.dma_start(b2sb[:], b2_d[:]))
            add(-4.6, lambda: emit_w_piece(w2x, w2_d, 0, DIMS[0][1] // P, 0, DIMS[0][2], "w2"))

            for i in range(1, nch):
                add(T1[i], (lambda j: (lambda: emit_xt(j)))(i))

            for e in range(1, E):
                d_in, d_hid, d_out = DIMS[e]
                nk1 = d_in // P
                nk2 = d_hid // P
                i0 = first_chunk_of[e]
                cn_first = chunks[i0][2]
                for (k0, k1, lo, hi, mfirst) in plan_w(e, 1):
                    need = T1[i0] + mfirst * nk1 * _sp(cn_first)
                    add(need, (lambda a, b_, c, d_: (lambda: emit_w_piece(w1x, w1_d, a, b_, c, d_, "w1")))(k0, k1, lo, hi))
                for (k0, k1, lo, hi, mfirst) in plan_w(e, 2):
                    need = T2[i0] + mfirst * nk2 * _sp(cn_first)
                    add(need, (lambda a, b_, c, d_: (lambda: emit_w_piece(w2x, w2_d, a, b_, c, d_, "w2")))(k0, k1, lo, hi))

            for _, _, fn in sorted(items, key=lambda it: (it[0], it[1])):
                fn()

            # ACT gelu table preload: issued after the Scalar-queue bootstrap
            # DMAs, with no data deps, so the ~1.3us table load overlaps the
            # first weight DMA instead of serializing before the first gelu.
            nc.scalar.activation(wact[:], wu[:], Gelu, bias=0.0)

            def wslice(xdict, k, m):
                """[128, 128] lhsT slice for feature cols [m*128,(m+1)*128)."""
                lo_c, hi_c = m * P, (m + 1) * P
                for lo, hi, k0, t_ in xdict[k]:
                    if lo <= lo_c and hi_c <= hi:
                        return t_[:, k - k0, lo_c - lo : hi_c - lo]
                raise AssertionError("weight slice not found")

            # ---- compute: 1-deep software pipeline over chunks ----
            # ht ping-pong buffers sized for the worst adjacent chunk pair
            htA = hpool.tile([P, H // P, CHUNK_W], bf16, tag="htA")  # holds e3 (32 slabs)
            htB = hpool.tile([P, H // P // 2, CHUNK_W], bf16, tag="htB")  # holds e2 (16)

            def mm1_block(i):
                e, c0, cn = chunks[i]
                d_in, d_hid, _ = DIMS[e]
                nk1, nm1 = d_in // P, d_hid // P
                ht = htA if i % 2 == 0 else htB
                xt = xts[(e, c0)]
                for m in range(nm1):
                    ps = pspool.tile([P, cn], f32, tag="ps")
                    for k in range(nk1):
                        nc.tensor.matmul(
                            ps[:],
                            wslice(w1x, k, m),
                            xt[:, k, :],
                            start=(k == 0),
                            stop=(k == nk1 - 1),
                        )
                    nc.scalar.activation(ht[:, m, :cn], ps[:], Gelu, bias=b1sb[:, m : m + 1])

            def mm2_block(i):
                e, c0, cn = chunks[i]
                _, d_hid, d_out = DIMS[e]
                nk2, nm2 = d_hid // P, d_out // P
                ht = htA if i % 2 == 0 else htB
                col = offs[e] + c0
                for m2 in range(nm2):
                    ps = pspool.tile([P, cn], f32, tag="ps")
                    for k2 in range(nk2):
                        nc.tensor.matmul(
                            ps[:],
                            wslice(w2x, k2, m2),
                            ht[:, k2, :cn],
                            start=(k2 == 0),
                            stop=(k2 == nk2 - 1),
                        )
                    # bias-add evicts PSUM to a bf16 SBUF slab; each slab
                    # streams out as soon as it's ready
                    yt = ypool.tile([P, cn], bf16, tag="yt")
                    nc.vector.tensor_scalar_add(yt[:], ps[:], b2sb[:, m2 : m2 + 1])
                    # outputs go out on the Scalar HW-DGE queue: it is idle
                    # during mm2 phases, and this keeps the input queue
                    # (Sync) purely need-ordered
                    nc.scalar.dma_start(y_d[:, m2, col : col + cn], yt[:])

            warm_mms(24)
            mm1_block(0)
            warm_mms(12)
            for i in range(1, nch):
                mm1_block(i)
                mm2_block(i - 1)
            mm2_block(nch - 1)

    nc.compile()
    return nc, ctot, offs


def _ensure_ntff_hook_importable():
    """bass_utils' trace path imports antenv.axon_hooks, which some images
    lack; install a working shim so tracing (e.g. BASS_TRACE=1 in the
    environment) degrades gracefully instead of crashing. No-op when the
    real module exists."""
    try:
        import antenv.axon_hooks  # noqa: F401
        return
    except ImportError:
        pass
    holder = {"hook": None}
    m = types.ModuleType("antenv.axon_hooks")
    m.set_axon_ntff_profile_hook = lambda h: holder.__setitem__("hook", h)
    m.get_axon_ntff_profile_hook = lambda: holder["hook"]
    sys.modules["antenv.axon_hooks"] = m
    try:
        from trn_agent_boot.trn_boot import _ntff_profile_via_ctypes

        m.set_axon_ntff_profile_hook(_ntff_profile_via_ctypes("/opt/axon/libaxon_pjrt.so"))
    except Exception:
        pass  # hook stays None; bass_utils logs and skips tracing


def kernel(x, expert_mask, w1, b1, w2, b2):
    _ensure_ntff_hook_importable()
    from concourse.bass_utils import run_bass_kernel_spmd

    B, N, _ = x.shape
    T = B * N
    xf = np.asarray(x, dtype=np.float32).reshape(T, D)
    mask = np.asarray(expert_mask).reshape(T).astype(np.int64)

    # --- host routing ---
    ids_by_e = [np.nonzero(mask == e)[0] for e in range(E)]
    counts = [len(i) for i in ids_by_e]
    caps = [max(8, _round_up(math.ceil(c / NCORES), 8)) for c in counts]
    # per (core, expert) token id arrays
    core_ids = [[None] * E for _ in range(NCORES)]
    for e in range(E):
        parts = np.array_split(ids_by_e[e], NCORES)
        for c in range(NCORES):
            assert len(parts[c]) <= caps[e]
            core_ids[c][e] = parts[c]

    nc, ctot, offs = _build_graph(caps)

    # --- host input prep ---
    w1t = _tile_fmajor(np.asarray(w1, np.float32).T).astype(BF16)  # [128, 8, H]
    w2t = _tile_fmajor(np.asarray(w2, np.float32).T).astype(BF16)  # [128, 32, OUT]
    b1t = np.ascontiguousarray(np.asarray(b1, np.float32).reshape(H // P, P).T)
    b2t = np.ascontiguousarray(np.asarray(b2, np.float32).reshape(OUT // P, P).T)

    in_maps = []
    for c in range(NCORES):
        xg = np.zeros((ctot, D), np.float32)
        for e in range(E):
            ids = core_ids[c][e]
            xg[offs[e] : offs[e] + len(ids)] = xf[ids]
        xt = _tile_fmajor(xg.T).astype(BF16)  # [128, 8, ctot]
        in_maps.append({"xt": xt, "w1t": w1t, "w2t": w2t, "b1t": b1t, "b2t": b2t})

    res = run_bass_kernel_spmd(nc, in_maps, list(range(NCORES)))

    # --- host output assembly ---
    y = np.zeros((T, OUT), np.float32)
    for c in range(NCORES):
        yr = np.asarray(res.results[c]["yt"]).astype(np.float32)  # [128, 8, ctot]
        yfull = yr.transpose(1, 0, 2).reshape(OUT, ctot)
        for e in range(E):
            d_out = DIMS[e][2]
            ids = core_ids[c][e]
            if len(ids):
                y[ids, :d_out] = yfull[:d_out, offs[e] : offs[e] + len(ids)].T
    return y.reshape(B, N, OUT)
